# revision 4
# baseline (speedup 1.0000x reference)
"""Multi-head attention (axis-swapped variant) on 8 Trainium2 NeuronCores.

64 effective heads of size 16 (head h owns contiguous projection columns
[16h, 16h+16)), causal softmax scaled by 1/sqrt(16), projections Wq/Wk/Wv,
output projection Wo + bo.

Sharding: core c = 4*b + g handles batch b and head-group g (16 heads = 256
projection columns). Each core returns a partial output [1024, 1024] bf16;
the host sums the 4 group partials per batch and adds bo.

Device pipeline (all matmuls fp8e4 DoubleRow except the fp32r out-proj):
  1. Q/K/V projections from fp8 x and W (4 accumulation steps of 2x128).
  2. Scores per (head, key-block): K-slice [8,2,128] x Q-slice [8,2,N].
  3. exp: split across ACT (true exp), DVE/Pool (Schraudolph int8 bitcast
     fp8e4m3, round-to-nearest verified on HW). Causal masking is fused into
     the DVE/Pool path via a host-built TRI bias tile (-1e9 -> int8 saturates
     to -128 = fp8 -0.0).
  4. ctx per (head, key-block-pair): VA [128,2,17] (16 v-cols + ones for the
     softmax denominator) x AT [128,2,N] in one DoubleRow matmul.
  5. Denominator reciprocal + broadcast (DRAM roundtrip), compact the 16 real
     rows per head via SBUF-SBUF DMA, normalize, fp32r out-projection.
"""

import numpy as np
import ml_dtypes

import concourse.bass as bass
import concourse.mybir as mybir
import concourse.tile as tile
from concourse.bass_utils import run_bass_kernel_spmd

F32 = mybir.dt.float32
F32R = mybir.dt.float32r
BF16 = mybir.dt.bfloat16
FP8 = mybir.dt.float8e4
I8 = mybir.dt.int8
I16 = mybir.dt.int16
E4M3 = ml_dtypes.float8_e4m3

EMB = 1024
SEQ = 1024
BATCH = 2
NG = 4            # head groups (cores per batch)
HPG = 16          # heads per group/core
DH = 16           # per-head feature size
GCOLS = HPG * DH  # 256 projection columns per core

A_SCH = float(128.0 * np.log2(np.e) * 0.25)  # bf16 schraudolph (folds 1/4)
B_SCH = 16247.7                              # calibrated for round-to-nearest
MASK_NEG = -1.0e9

Exp = mybir.ActivationFunctionType.Exp
Copy = mybir.ActivationFunctionType.Copy
MULT = None  # set after mybir import below
DR = mybir.MatmulPerfMode.DoubleRow


def split_excess_waits(nc, cap=1):
    """Walrus rejects instructions carrying more than a few semaphore waits.
    Relocate excess waits onto preceding same-engine EventSemaphore
    instructions."""

    def fix_block(bb, dummy):
        insts = bb.instructions
        i = 0
        while i < len(insts):
            inst = insts[i]
            si = inst.sync_info
            waits = list(si.on_wait) if si is not None and si.on_wait else []
            if len(waits) > cap:
                eng = nc.engines[inst.engine]
                excess, keep = waits[:-cap], waits[-cap:]
                si.on_wait = keep
                pos = i
                for j in range(0, len(excess), cap):
                    chunk = excess[j : j + cap]
                    ev = eng.wait_ge(dummy, 1)
                    cur_list = nc.cur_bb.bb.instructions
                    assert cur_list[-1] is ev.ins
                    cur_list.pop()
                    ev.ins.sync_info.on_wait = chunk
                    insts.insert(pos, ev.ins)
                    pos += 1
                    i += 1
            i += 1

    with nc.semaphore("waitfix_dummy") as dummy:
        for f in nc.m.functions:
            for bb in f.blocks:
                fix_block(bb, dummy)


def build_nc():
    nc = bass.Bass()
    mult = mybir.AluOpType.mult
    add = mybir.AluOpType.add

    xt_d = nc.declare_dram_parameter("xt8", [128, 8, SEQ], BF16, isOutput=False)
    wq_d = nc.declare_dram_parameter("wq8", [128, 8, GCOLS], BF16, isOutput=False)
    wk_d = nc.declare_dram_parameter("wk8", [128, 8, GCOLS], BF16, isOutput=False)
    wv_d = nc.declare_dram_parameter("wv8", [128, 8, GCOLS], BF16, isOutput=False)
    wo_d = nc.declare_dram_parameter("wo", [128, 2, EMB], F32R, isOutput=False)
    tri_d = nc.declare_dram_parameter("tri", [128, 2, 512], F32, isOutput=False)
    y_d = nc.declare_dram_parameter("y", [SEQ, EMB], BF16, isOutput=True)
    import os as _os2
    DBG = _os2.environ.get("KDBG", "") == "1"
    if DBG:
        dqs_d = nc.declare_dram_parameter("dqs", [128, 2, SEQ], mybir.dt.int8, isOutput=True)
        dscr_d = nc.declare_dram_parameter("dscr", [128, 512], F32, isOutput=True)
        db2_d = nc.declare_dram_parameter("db2", [64, 512], F32, isOutput=True)
        dat_d = nc.declare_dram_parameter("dat", [128, 2, 512], mybir.dt.int16, isOutput=True)
        dqf_d = nc.declare_dram_parameter("dqf", [128, 4, 2, SEQ], mybir.dt.int8, isOutput=True)

    # --- static engine load balancer (ns-weighted) ---
    rate = {"act": 0.8333, "dve": 1.0417, "pool": 1.389}
    over = {"act": 150.0, "dve": 130.0, "pool": 100.0}
    load = {"act": 0.0, "dve": 0.0, "pool": 0.0}

    def pick(cols, allowed):
        e = min(allowed, key=lambda k: load[k] + cols * rate[k] + over[k])
        load[e] += cols * rate[e] + over[e]
        return e

    with tile.TileContext(nc) as tc:
        with (
            tc.tile_pool(name="big", bufs=1) as big,
            tc.tile_pool(name="work", bufs=6) as work,
            tc.tile_pool(name="att", bufs=14) as att,
            tc.tile_pool(name="dram", bufs=1, space="DRAM") as dram,
        ):
            def emit_copy(dst, src, cols, allowed=("act", "dve")):
                e = pick(cols, allowed)
                if e == "act":
                    nc.scalar.activation(dst, src, Copy)
                elif e == "dve":
                    nc.vector.tensor_copy(dst, src)
                else:
                    nc.gpsimd.tensor_copy(dst, src)

            drc = dram.tile([2, 4, 4, 512], F32)  # (chunk, class, g, q) denoms
            drr = dram.tile([2, 4, 4, 512], F32)  # reciprocals

            # ---- load everything ----
            XT = big.tile([128, 8, SEQ], BF16)
            for kb in range(8):
                nc.sync.dma_start(XT[:, kb, :], xt_d[:, kb, :])
            WQ = big.tile([128, 8, GCOLS], BF16)
            nc.sync.dma_start(WQ[:], wq_d[:])
            WK = big.tile([128, 8, GCOLS], BF16)
            nc.sync.dma_start(WK[:], wk_d[:])
            WV = big.tile([128, 8, GCOLS], BF16)
            nc.sync.dma_start(WV[:], wv_d[:])
            WO = big.tile([128, 2, EMB], F32R)
            nc.sync.dma_start(WO[:], wo_d[:])
            TRI = big.tile([128, 2, 512], F32)
            nc.sync.dma_start(TRI[:], tri_d[:])

            # PE warmup: junk matmuls to ramp the p-state while DMAs land.
            JW = big.tile([128, 512], BF16)
            nc.gpsimd.memset(JW[:], 0.0)
            with tc.tile_pool(name="ps_w", bufs=1, space="PSUM") as ps_w:
                jp = ps_w.tile([128, 512], F32)
                for _ in range(8):
                    nc.tensor.matmul(
                        jp[:], JW[:, 0:128], JW[:], start=True, stop=True,
                    )

            # ---- projections (fp8 DoubleRow) ----
            QS = big.tile([128, 2, SEQ], FP8)   # staging: part=32g+8a+f8, (i, m)
            KS = big.tile([128, 2, SEQ], FP8)
            VA = big.tile([128, 4, HPG, 2, 32], BF16)
            nc.gpsimd.memset(VA[:, :, :, :, 16:17], 1.0)
            nc.gpsimd.memset(VA[:, :, :, :, 17:32], 0.0)

            with tc.tile_pool(name="ps_proj", bufs=2, space="PSUM") as ps_proj:
                for W, S in ((WK, KS), (WQ, QS)):
                    for t in range(2):
                        for ic in range(2):
                            pq = ps_proj.tile([128, 512], F32, tag="pproj")
                            for kb in range(8):
                                nc.tensor.matmul(
                                    pq[:],
                                    W[:, kb, 128 * t : 128 * t + 128],
                                    XT[:, kb, 512 * ic : 512 * ic + 512],
                                    start=(kb == 0),
                                    stop=(kb == 7),
                                )
                            emit_copy(
                                S[:, t, 512 * ic : 512 * ic + 512], pq[:], 512
                            )
            # shuffle Q/K staging into alignment tiles: host col order puts
            # half-feature i of head H=4g+a at staging partition 32g+8a+f8,
            # so class a is a uniform 8a-partition shift (one 3-dim DMA).
            if DBG:
                nc.sync.dma_start(dqs_d[:], QS[:].bitcast(I8))
            QF = big.tile([128, 4, 2, SEQ], FP8, name="qf")
            KF = big.tile([128, 4, 2, SEQ], FP8, name="kf")
            for S, T in ((QS, QF), (KS, KF)):
                for a in range(4):
                    for g in range(4):
                        nc.sync.dma_start(
                            T[32 * g : 32 * g + 8, a, :, :],
                            S[32 * g + 8 * a : 32 * g + 8 * a + 8, :, :],
                        )

            # ---- attention ----
            if DBG:
                nc.sync.dma_start(dqf_d[:], QF[:].bitcast(I8))
            SC2N = big.tile([128, 2, SEQ], F32R)  # normalized compact ctx
            import os as _os
            ABL = _os.environ.get("ABL", "")
            AT0 = None
            if ABL == "skeleton":
                AT0 = big.tile([128, 2, 512], BF16, name="at0")
                nc.gpsimd.memset(AT0[:], 0.0)
            with (
                tc.tile_pool(name="ps_sw", bufs=6, space="PSUM") as ps_sw,
                tc.tile_pool(name="ps_cp", bufs=2, space="PSUM") as ps_cp,
            ):
                for mt in range(8):
                    pv = ps_cp.tile([128, GCOLS], F32, tag="cp")
                    for kb in range(8):
                        nc.tensor.matmul(
                            pv[:],
                            XT[:, kb, 128 * mt : 128 * mt + 128],
                            WV[:, kb, :],
                            start=(kb == 0),
                            stop=(kb == 7),
                        )
                    emit_copy(
                        VA[:, mt // 2, :, mt % 2, 0:16],
                        pv[:].rearrange("p (h e) -> p h e", e=DH),
                        256,
                    )

                from collections import deque

                pend_ctx = deque()
                CTX_LAG = 5

                def attn_group(a, ic):
                    c0 = 512 * ic
                    npair = 2 * (ic + 1)
                    CP = ps_cp.tile([128, 512], F32, tag="cp")
                    for g in range(4):
                        H = 4 * g + a
                        for pp in range(npair):
                            s = max(0, 256 * pp - c0)
                            N = 512 - s
                            diag = 256 * pp >= c0
                            AT = att.tile([128, 2, 512], BF16, tag="at")
                            for i in range(2):
                                j = 2 * pp + i
                                # slot 1 of a diag pair: skip the fully-masked
                                # first 128 query cols (Pool memsets them)
                                st = 128 if (diag and i) else 0
                                SW = ps_sw.tile([128, 512], F32, tag="sw")
                                nc.tensor.matmul(
                                    SW[:, 0 : N - st],
                                    KF[32 * g : 32 * g + 8, a, :,
                                       128 * j : 128 * j + 128],
                                    QF[32 * g : 32 * g + 8, a, :,
                                       c0 + s + st : c0 + 512],
                                    start=True,
                                    stop=True,
                                    perf_mode=DR,
                                    tile_position=(32 * g, 0),
                                )
                                if ABL == "skeleton":
                                    continue
                                if diag and ABL != "nodiag":
                                    # masked region: slot 0 triangle [s, s+128);
                                    # slot 1 strip+triangle [s, s+256)
                                    mw = 128 * (i + 1)
                                    dve_c = load["dve"] + N * rate["dve"]
                                    alt_c = max(
                                        load["act"]
                                        + (N - mw + 128) * rate["act"],
                                        load["pool"]
                                        + mw * rate["pool"]
                                        + (over["pool"] if i else 0.0),
                                    )
                                    if dve_c <= alt_c:
                                        load["dve"] += (
                                            (N - st) * rate["dve"]
                                            + over["dve"]
                                        )
                                        nc.vector.scalar_tensor_tensor(
                                            AT[:, i, st:N].bitcast(I16),
                                            SW[:, 0 : N - st],
                                            A_SCH,
                                            TRI[:, i, st:N],
                                            mult,
                                            add,
                                        )
                                        if i:
                                            load["pool"] += (
                                                128 * rate["pool"]
                                                + over["pool"]
                                            )
                                            nc.gpsimd.memset(
                                                AT[:, i, 0:128], 0.0
                                            )
                                    else:
                                        # ACT exp skips the fully-masked strip
                                        load["act"] += (
                                            (N - st) * rate["act"]
                                            + over["act"]
                                        )
                                        nc.scalar.activation(
                                            AT[:, i, st:N], SW[:, 0 : N - st],
                                            Exp, scale=0.25,
                                        )
                                        if i:
                                            load["pool"] += (
                                                128 * rate["pool"]
                                                + over["pool"]
                                            )
                                            nc.gpsimd.memset(
                                                AT[:, i, 0:128], 0.0
                                            )
                                        # zero the triangle (keep k >= j)
                                        load["pool"] += (
                                            128 * rate["pool"] + over["pool"]
                                        )
                                        nc.gpsimd.affine_select(
                                            out=AT[:, i, st : st + 128],
                                            in_=AT[:, i, st : st + 128],
                                            compare_op=mybir.AluOpType.is_ge,
                                            fill=0.0,
                                            base=0,
                                            pattern=[[1, 128]],
                                            channel_multiplier=-1,
                                        )
                                else:
                                    e = pick(
                                        N,
                                        ("act",) if ABL == "actonly"
                                        else ("act", "dve"),
                                    )
                                    if e == "act":
                                        nc.scalar.activation(
                                            AT[:, i, 0:N], SW[:, 0:N],
                                            Exp, scale=0.25,
                                        )
                                    else:
                                        nc.vector.tensor_scalar(
                                            AT[:, i, 0:N].bitcast(I16),
                                            SW[:, 0:N],
                                            A_SCH,
                                            B_SCH,
                                            mult,
                                            add,
                                        )
                            def ctx_mm(CP=CP, g=g, s=s, H=H, pp=pp, AT=AT,
                                       N=N, npair=npair):
                                for i in range(2):
                                    nc.tensor.matmul(
                                        CP[32 * g : 32 * g + 32, s:512],
                                        VA[:, pp, H, i, :],
                                        (AT0 if ABL == "skeleton" else AT)[
                                            :, i, 0:N
                                        ],
                                        start=(pp == 0 and i == 0),
                                        stop=(pp == npair - 1 and i == 1),
                                        skip_group_check=True,
                                        tile_position=(0, 32 * g),
                                    )

                            if DBG and a == 0 and ic == 0 and g == 0 and pp == 0:
                                nc.sync.dma_start(dat_d[:], AT[:].bitcast(I16))
                            pend_ctx.append(ctx_mm)
                            if len(pend_ctx) > CTX_LAG:
                                pend_ctx.popleft()()
                    # evacuate ctx for this (class, chunk)
                    while pend_ctx:
                        pend_ctx.popleft()()
                    SCr = work.tile([128, 512], F32, tag="scr")
                    emit_copy(SCr[:], CP[:], 512)
                    sr = SCr[:]
                    if DBG and a == 0 and ic == 0:
                        nc.sync.dma_start(dscr_d[:], SCr[:])
                    # compact the 16 real rows per head: SBUF->SBUF DMA
                    kk, pb = divmod(a, 2)
                    SC2 = work.tile([64, 512], F32, tag="sc2")
                    for g in range(4):
                        nc.sync.dma_start(
                            SC2[16 * g : 16 * g + 16, :],
                            SCr[32 * g : 32 * g + 16, :],
                        )
                    # denominators: gather, reciprocal, then broadcast
                    D4 = work.tile([4, 512], F32, tag="d4")
                    nc.sync.dma_start(
                        D4[:],
                        bass.AP(
                            tensor=sr.tensor,
                            offset=sr[16:17].offset,
                            ap=[[32 * sr.ap[0][0], 4]] + sr[16:17].ap[1:],
                        ),
                    )
                    R4 = work.tile([4, 512], F32, tag="r4")
                    nc.vector.reciprocal(out=R4[:], in_=D4[:])
                    load["dve"] += 512 * rate["dve"] + over["dve"]
                    nc.sync.dma_start(drr[ic, a, :, :], R4[:])
                    B2 = work.tile([64, 512], F32, tag="b2")
                    for g in range(4):
                        nc.sync.dma_start(
                            B2[16 * g : 16 * g + 16, :],
                            drr[ic, a, g : g + 1, :].to_broadcast([16, 512]),
                        )
                    load["dve"] += 512 * rate["dve"] + over["dve"]
                    if DBG and a == 0 and ic == 0:
                        nc.sync.dma_start(db2_d[:], B2[:])
                    nc.vector.tensor_tensor(
                        SC2N[64 * pb : 64 * pb + 64, kk, c0 : c0 + 512],
                        SC2[:],
                        B2[:],
                        mybir.AluOpType.mult,
                    )

                def out_proj(ic):
                    for ib in range(4 * ic, 4 * ic + 4):
                        Y = work.tile([128, EMB], BF16, tag="y")
                        for oc in range(2):
                            po = ps_sw.tile([128, 512], F32, tag="sw")
                            for kk in range(2):
                                nc.tensor.matmul(
                                    po[:],
                                    SC2N[:, kk,
                                         128 * ib : 128 * ib + 128],
                                    WO[:, kk, 512 * oc : 512 * oc + 512],
                                    start=(kk == 0),
                                    stop=(kk == 1),
                                )
                            emit_copy(
                                Y[:, 512 * oc : 512 * oc + 512], po[:], 512
                            )
                        nc.sync.dma_start(
                            y_d[128 * ib : 128 * ib + 128, :], Y[:]
                        )

                for a in range(4):
                    attn_group(a, 0)
                    attn_group(a, 1)
                out_proj(0)
                out_proj(1)
    split_excess_waits(nc)
    return nc


_NC_CACHE = None


def _get_nc():
    global _NC_CACHE
    if _NC_CACHE is None:
        _NC_CACHE = build_nc()
    return _NC_CACHE


def _host_inputs(x, Wq, Wk, Wv, Wo):
    """Per-core input prep (fp8 casts + layout permutations)."""
    tri = np.full((128, 2, 512), np.float32(MASK_NEG), dtype=np.float32)
    k = np.arange(512)[None, None, :]
    j = np.arange(128)[:, None, None]
    i = np.arange(2)[None, :, None]
    np.copyto(tri, np.float32(B_SCH), where=(k >= 128 * i + j))

    in_maps = []
    for c in range(8):
        b, g = divmod(c, NG)
        cols = slice(GCOLS * g, GCOLS * g + GCOLS)
        # xT: [p, kb, m] with d_in = 128*kb + p
        xt = np.ascontiguousarray(
            x[b].T.reshape(8, 128, SEQ).transpose(1, 0, 2)
        ).astype(ml_dtypes.bfloat16)

        iidx = np.arange(GCOLS)
        i_, r = divmod(iidx, 128)
        g_, r2 = divmod(r, 32)
        a_, f8 = divmod(r2, 8)
        qk_perm = 16 * (4 * g_ + a_) + 8 * i_ + f8

        def wsplit(W, perm=True):
            # [d_in, col'] -> [p, kb, col']; col' = 128*i + 32*g + 8*a + f8
            # holds feature 8i+f8 of head H = 4g + a (perm=False: natural cols)
            Wg = W[:, cols]
            if perm:
                Wg = Wg[:, qk_perm]
            return np.ascontiguousarray(
                Wg.reshape(8, 128, GCOLS).transpose(1, 0, 2)
            ).astype(ml_dtypes.bfloat16)

        # wo rows: r = 64*(H%4) + 16*(H//4) + e  <- orig row 16H + e
        wo_p = np.empty((256, EMB), dtype=np.float32)
        wog = Wo[cols, :]
        for H in range(HPG):
            r = 64 * (H % 4) + 16 * (H // 4)
            wo_p[r : r + 16] = wog[16 * H : 16 * H + 16]
        wo_t = np.ascontiguousarray(wo_p.reshape(2, 128, EMB).transpose(1, 0, 2))

        in_maps.append(
            {
                "xt8": xt,
                "wq8": wsplit(Wq),
                "wk8": wsplit(Wk),
                "wv8": wsplit(Wv, perm=False),
                "wo": wo_t,
                "tri": tri,
            }
        )
    return in_maps


def kernel(x, Wq, Wk, Wv, Wo, bo):
    x = np.asarray(x, dtype=np.float32)
    Wq = np.asarray(Wq, dtype=np.float32)
    Wk = np.asarray(Wk, dtype=np.float32)
    Wv = np.asarray(Wv, dtype=np.float32)
    Wo = np.asarray(Wo, dtype=np.float32)
    bo = np.asarray(bo, dtype=np.float32)

    nc = _get_nc()
    in_maps = _host_inputs(x, Wq, Wk, Wv, Wo)
    res = run_bass_kernel_spmd(nc, in_maps, core_ids=list(range(8)))
    out = np.zeros((BATCH, SEQ, EMB), dtype=np.float32)
    for c in range(8):
        b = c // NG
        out[b] += res.results[c]["y"].astype(np.float32)
    out += bo[None, None, :]
    return out
